# revision 1
# baseline (speedup 1.0000x reference)
"""ANFIS forward kernel for Trainium2, 8-core data-parallel.

Model (reference):
    mf        = exp(-(x-c)^2 / (2 s^2))            # (N,R,D)
    strengths = prod_d mf                          # (N,R)
    norm      = strengths / (sum_r strengths + eps)
    wi        = X @ coeffs[r,:-1] + coeffs[r,-1]   # (N,R,O)
    out       = softmax_o( sum_r norm * wi )       # (N,O)

Kernel algebra (per row n):
    l_r   = sum_d [ 2*c*a*x - a*x^2 ] - k_r,   a = 1/(2 s^2), k_r = sum_d c^2 a
    s_r   = exp(l_r)                            (strengths)
    G_io  = sum_r s_r * Chat[r,i,o]             (Chat = [coeffs[:, :16]; bias row; ones])
    U_o   = sum_i xhat_i * G_io                 (xhat = [x, 1])
    S     = G[:,170] = sum_r s_r
    out   = softmax_o( U / (S + eps) )          (no max-subtraction needed; |U/S| small)

Layout: rows are processed in groups of 512 (4 tiles x 128 partitions).
  - One PE transpose turns 4 natural tiles (128,16)x{x,x^2} into xs4 (128,128)
    with partition = (s, j, d).
  - M1 (fp32): l4 (128=(j,r), 128 rows) via block-diagonal weight WL4.
  - exp on ACT with per-partition bias -k_r -> sst4.
  - M2 (f32r): G4 (128 rows, 4 x 256) via block-diagonal Chat, data-stationary.
  - DVE: P4 = G4 * xhat (broadcast over o), strided reduce over i -> U.
  - Softmax batched over 4 groups on DVE/ACT.
"""

import numpy as np

N, D, R, O = 131072, 16, 32, 10
EPS = 1e-8
NCORES = 8
MC = N // NCORES          # rows per core = 16384
TPG = 4                   # tiles (of 128 rows) per group
GROUP = 128 * TPG         # 512 rows per group
NG = MC // GROUP          # 32 groups per core
META = 4                  # groups per softmax batch
NMETA = NG // META        # 8

DI = D + 1                # 17: x dims + ones
F = DI * O                # 170 product features
FS = F + 1                # 171: + strength-sum feature
FPAD = 256                # per-tile feature stride in G4 (bank alignment)


def _build_constants(centers, sigmas, coeffs):
    a = 1.0 / (2.0 * sigmas.astype(np.float64) ** 2)          # (R,D)
    c = centers.astype(np.float64)

    # WL4: lhsT for M1. out partition (j,r) = j*32+r ; rhs partition (s,j,d).
    wl4 = np.zeros((128, 128), np.float64)
    for j in range(TPG):
        for r in range(R):
            pi = j * R + r
            for d in range(D):
                wl4[0 * 64 + j * 16 + d, pi] = 2.0 * c[r, d] * a[r, d]   # x part
                wl4[1 * 64 + j * 16 + d, pi] = -a[r, d]                  # x^2 part
    negk = -(c * c * a).sum(axis=1)                            # (R,)
    negk4 = np.tile(negk, TPG).reshape(128, 1)

    # Chat (R, 171): features f = i*10+o (i=16 -> bias row), f=170 -> ones.
    chat = np.zeros((R, FS), np.float64)
    chat[:, :F] = coeffs.astype(np.float64).reshape(R, FS - 1)  # (R,17*10)
    chat[:, F] = 1.0
    # C2D4 (128, 1024): [(j,r), j'*256+f] = delta_jj' * chat[r,f]
    c2d4 = np.zeros((128, TPG * FPAD), np.float64)
    for j in range(TPG):
        c2d4[j * R:(j + 1) * R, j * FPAD:j * FPAD + FS] = chat
    return (wl4.astype(np.float32), negk4.astype(np.float32),
            c2d4.astype(np.float32))


def _build_bass():
    import concourse.bacc as bacc
    import concourse.mybir as mybir
    from concourse import masks
    from concourse.tile import TileContext

    f32 = mybir.dt.float32
    f32r = mybir.dt.float32r
    AX = mybir.AxisListType
    ALU = mybir.AluOpType
    ACTF = mybir.ActivationFunctionType

    nc = bacc.Bacc("TRN2", target_bir_lowering=False, debug=False)
    xin = nc.declare_dram_parameter("xin", [MC, DI], f32, isOutput=False)
    wl4_d = nc.declare_dram_parameter("wl4", [128, 128], f32, isOutput=False)
    negk4_d = nc.declare_dram_parameter("negk4", [128, 1], f32, isOutput=False)
    c2d4_d = nc.declare_dram_parameter("c2d4", [128, TPG * FPAD], f32r,
                                       isOutput=False)
    yout = nc.declare_dram_parameter("yout", [MC, O], f32, isOutput=True)

    with TileContext(nc) as tc:
        with (
            tc.tile_pool(name="const", bufs=1) as cpool,
            tc.tile_pool(name="work", bufs=3) as wpool,
            tc.tile_pool(name="dmain", bufs=NG) as dpool,
            tc.tile_pool(name="stage", bufs=2) as spool,
            tc.tile_pool(name="ps_t", bufs=2, space="PSUM") as ps_t,
            tc.tile_pool(name="ps_l", bufs=2, space="PSUM") as ps_l,
            tc.tile_pool(name="ps_g", bufs=2, space="PSUM") as ps_g,
        ):
            ident = cpool.tile([128, 128], f32)
            masks.make_identity(nc, ident[:])
            wl4 = cpool.tile([128, 128], f32)
            nc.sync.dma_start(out=wl4[:], in_=wl4_d[:, :])
            negk4 = cpool.tile([128, 1], f32)
            nc.sync.dma_start(out=negk4[:], in_=negk4_d[:, :])
            c2d4 = cpool.tile([128, TPG * FPAD], f32r)
            nc.sync.dma_start(out=c2d4[:], in_=c2d4_d[:, :])

            for m in range(NMETA):
                u16 = spool.tile([128, META * TPG * O], f32, tag="u16")
                s16 = spool.tile([128, META * TPG], f32, tag="s16")
                e16 = spool.tile([128, META * TPG * O], f32, tag="e16")
                se16 = spool.tile([128, META * TPG], f32, tag="se16")
                o16 = spool.tile([128, META * TPG * O], f32, tag="o16")

                for q in range(META):
                    g = m * META + q
                    # -- load 512 rows of [X, 1] -> (128, (4,17)) -----------
                    xa = dpool.tile([128, TPG * DI], f32, tag="xa")
                    src = xin[g * GROUP:(g + 1) * GROUP, :].rearrange(
                        "(j p) c -> p j c", p=128)
                    nc.sync.dma_start(
                        out=xa[:].rearrange("p (j c) -> p j c", c=DI),
                        in_=src)
                    # -- contiguous x tile for the PE transpose ------------
                    xt_in = dpool.tile([128, TPG * D], f32, tag="xt_in")
                    nc.sync.dma_start(
                        out=xt_in[:].rearrange("p (j c) -> p j c", c=D),
                        in_=src[:, :, 0:D])
                    # -- transpose (j,d) ordinals -> partitions ------------
                    xtp = ps_t.tile([TPG * D, 128], f32, tag="xtp")
                    nc.tensor.transpose(xtp[:], xt_in[:], ident[:])
                    # -- xs4 = [xT ; xT^2] via ACT --------------------------
                    xs4 = wpool.tile([128, 128], f32, tag="xs4")
                    nc.scalar.activation(xs4[0:TPG * D, :], xtp[:], ACTF.Copy)
                    nc.scalar.activation(xs4[TPG * D:128, :], xtp[:],
                                         ACTF.Square)
                    # -- M1: logits ----------------------------------------
                    l4 = ps_l.tile([128, 128], f32, tag="l4")
                    nc.tensor.matmul(l4[:], lhsT=wl4[:], rhs=xs4[:],
                                     start=True, stop=True)
                    # -- strengths = exp(l - k) ----------------------------
                    sst4 = wpool.tile([128, 128], f32r, tag="sst4")
                    nc.scalar.activation(sst4[:], l4[:], ACTF.Exp,
                                         bias=negk4[:, 0:1], scale=1.0)
                    # -- M2: G = strengths-weighted consequent sums --------
                    g4 = ps_g.tile([128, TPG * FPAD], f32, tag="g4")
                    nc.tensor.matmul(
                        g4[:, 0:512], lhsT=sst4[:],
                        rhs=c2d4[:, 0:512],
                        start=True, stop=True)
                    nc.tensor.matmul(
                        g4[:, 512:1024], lhsT=sst4[:],
                        rhs=c2d4[:, 512:1024],
                        start=True, stop=True)
                    # -- P = G * xhat (bcast over o) -----------------------
                    p4 = wpool.tile([128, TPG * F], f32, tag="p4")
                    p4v = p4[:].rearrange("p (j i o) -> p j i o",
                                          j=TPG, i=DI)
                    g4v = g4[:].rearrange("p (j f) -> p j f",
                                          j=TPG)[:, :, 0:F].rearrange(
                        "p j (i o) -> p j i o", i=DI)
                    xhv = xa[:, 0:TPG * DI].rearrange(
                        "p (j i) -> p j i", i=DI).unsqueeze(3).broadcast_to(
                        [128, TPG, DI, O])
                    nc.vector.tensor_tensor(p4v, g4v, xhv, ALU.mult)
                    # -- S extract + U = sum_i P ---------------------------
                    nc.vector.tensor_copy(
                        s16[:, q * TPG:(q + 1) * TPG],
                        g4[:].rearrange("p (j f) -> p j f",
                                        j=TPG)[:, :, F:F + 1].squeeze(2))
                    nc.vector.tensor_reduce(
                        u16[:, q * TPG * O:(q + 1) * TPG * O].rearrange(
                            "p (j o) -> p j o", j=TPG),
                        p4v.transpose([0, 1, 3, 2]),
                        axis=AX.X, op=ALU.add)

                # -- batched normalize + softmax over 16 tiles -------------
                nc.vector.tensor_scalar_add(s16[:], s16[:], EPS)
                nc.vector.reciprocal(s16[:], s16[:])
                u16v = u16[:].rearrange("p (g o) -> p g o", o=O)
                s16b = s16[:].unsqueeze(2).broadcast_to(
                    [128, META * TPG, O])
                nc.vector.tensor_tensor(u16v, u16v, s16b, ALU.mult)
                nc.scalar.activation(e16[:], u16[:], ACTF.Exp)
                nc.vector.tensor_reduce(
                    se16[:], e16[:].rearrange("p (g o) -> p g o", o=O),
                    axis=AX.X, op=ALU.add)
                nc.vector.reciprocal(se16[:], se16[:])
                se16b = se16[:].unsqueeze(2).broadcast_to(
                    [128, META * TPG, O])
                nc.vector.tensor_tensor(
                    o16[:].rearrange("p (g o) -> p g o", o=O),
                    e16[:].rearrange("p (g o) -> p g o", o=O),
                    se16b, ALU.mult)
                # -- store 2048 rows ---------------------------------------
                dst = yout[m * META * GROUP:(m + 1) * META * GROUP,
                           :].rearrange("(q j p) o -> p q j o",
                                        q=META, j=TPG)
                nc.sync.dma_start(
                    out=dst,
                    in_=o16[:].rearrange("p (q j o) -> p q j o",
                                         q=META, j=TPG))
    nc.compile()
    return nc


_NC_CACHE = None


def kernel(X, centers, sigmas, coeffs):
    global _NC_CACHE
    from concourse import bass_utils

    X = np.asarray(X, np.float32)
    wl4, negk4, c2d4 = _build_constants(
        np.asarray(centers, np.float32),
        np.asarray(sigmas, np.float32),
        np.asarray(coeffs, np.float32))

    xaug = np.concatenate(
        [X, np.ones((N, 1), np.float32)], axis=1)          # (N, 17)

    if _NC_CACHE is None:
        _NC_CACHE = _build_bass()
    nc = _NC_CACHE

    in_maps = []
    for c in range(NCORES):
        in_maps.append({
            "xin": np.ascontiguousarray(xaug[c * MC:(c + 1) * MC]),
            "wl4": wl4, "negk4": negk4, "c2d4": c2d4,
        })
    res = bass_utils.run_bass_kernel_spmd(nc, in_maps, list(range(NCORES)))
    return np.concatenate([r["yout"] for r in res.results], axis=0)



# revision 4
# speedup vs baseline: 1.1987x; 1.1987x over previous
"""ANFIS forward kernel for Trainium2, 8-core data-parallel.

Model (reference):
    mf        = exp(-(x-c)^2 / (2 s^2))            # (N,R,D)
    strengths = prod_d mf                          # (N,R)
    norm      = strengths / (sum_r strengths + eps)
    wi        = X @ coeffs[r,:-1] + coeffs[r,-1]   # (N,R,O)
    out       = softmax_o( sum_r norm * wi )       # (N,O)

Kernel algebra (per row n):
    l_r   = sum_d [ 2*c*a*x - a*x^2 ] - k_r,   a = 1/(2 s^2), k_r = sum_d c^2 a
    s_r   = exp(l_r)                            (strengths)
    G_io  = sum_r s_r * Chat[r,i,o]             (Chat = [coeffs[:, :16]; bias row; ones])
    U_o   = sum_i xhat_i * G_io                 (xhat = [x, 1])
    S     = sum_r s_r  (via ones feature)
    out   = softmax_o( U / (S + eps) )

Layout: supergroups of 2048 rows = 16 tiles of 128 rows; j in 0..3 indexes
row-tiles within a 512-row group, q in 0..3 indexes groups in a supergroup.
  - Host supplies X pre-transposed (partition=(j,d)) so no PE transpose;
    ACT squares it into partitions 64..127.
  - M1 (f32r): one (128,512) matmul per supergroup -> logits l[(j,r),(q,p)].
  - exp on ACT with per-partition bias -k_r -> sst (f32r, SBUF).
  - M2 (f32r): per group, lhsT=sst column block, rhs=block-diag Chat with
    feature order f = o*17 + i (i innermost), padded to 256 per j-block.
  - DVE: P = G * xhat (broadcast over o, unit-stride i), reduce over i -> U.
  - Softmax batched over 16 tiles per supergroup.
All DRAM I/O is contiguous per partition; host does the (cheap) unpermutes.
"""

import numpy as np

N, D, R, O = 131072, 16, 32, 10
EPS = 1e-8
NCORES = 8
MC = N // NCORES          # rows per core = 16384
TPG = 4                   # tiles (of 128 rows) per group
GROUP = 128 * TPG         # 512 rows per group
NG = MC // GROUP          # 32 groups per core
META = 4                  # groups per supergroup
NMETA = NG // META        # 8 supergroups
SGC = META * 128          # matmul columns per supergroup (512)

DI = D + 1                # 17: x dims + ones
F = DI * O                # 170 product features
FS = F + 1                # 171: + strength-sum feature
FPAD = 256                # per-j feature stride in G (bank-friendly)


def _build_constants(centers, sigmas, coeffs):
    a = 1.0 / (2.0 * sigmas.astype(np.float64) ** 2)          # (R,D)
    c = centers.astype(np.float64)

    # WL: lhsT for M1. out partition (j,r) = j*32+r ; rhs partition (s,j,d).
    wl = np.zeros((128, 128), np.float64)
    for j in range(TPG):
        for r in range(R):
            pi = j * R + r
            for d in range(D):
                wl[0 * 64 + j * 16 + d, pi] = 2.0 * c[r, d] * a[r, d]   # x
                wl[1 * 64 + j * 16 + d, pi] = -a[r, d]                  # x^2
    negk = -(c * c * a).sum(axis=1)                            # (R,)
    negk4 = np.tile(negk, TPG).reshape(128, 1)

    # Chat (R, FS) with feature order f = o*17 + i  (i innermost), f=170 ones.
    chat = np.zeros((R, FS), np.float64)
    chat[:, :F] = (
        coeffs.astype(np.float64).transpose(0, 2, 1).reshape(R, F))  # (R,(o,i))
    chat[:, F] = 1.0
    # c2d (128, 4*FPAD): [(j,r), j*FPAD + f] = chat[r,f]
    c2d = np.zeros((128, TPG * FPAD), np.float64)
    for j in range(TPG):
        c2d[j * R:(j + 1) * R, j * FPAD:j * FPAD + FS] = chat
    return (wl.astype(np.float32), negk4.astype(np.float32),
            c2d.astype(np.float32))


def _prepare_x(X):
    """Per-core host-side layouts (pure permutation / padding)."""
    X = np.asarray(X, np.float32)
    # row = ((m*META + q)*TPG + j)*128 + p
    xv = X.reshape(NCORES, NMETA, META, TPG, 128, D)
    # XT[(j,d), m*512 + q*128 + p]
    xt = np.ascontiguousarray(
        xv.transpose(0, 3, 5, 1, 2, 4).reshape(NCORES, 64, NMETA * SGC))
    # xa[p, (m, q, j, i)] with ones at i=16
    xa = np.empty((NCORES, 128, NMETA, META, TPG, DI), np.float32)
    xa[..., :D] = xv.transpose(0, 4, 1, 2, 3, 5)
    xa[..., D] = 1.0
    xa = np.ascontiguousarray(xa.reshape(NCORES, 128, NG * TPG * DI))
    return xt, xa


def _unpermute_out(yts):
    """yts: list of (128, NG*TPG*O) per core -> (N, O)."""
    y = np.stack(yts, axis=0).reshape(NCORES, 128, NMETA, META, TPG, O)
    y = y.transpose(0, 2, 3, 4, 1, 5).reshape(N, O)
    return np.ascontiguousarray(y)


def _build_bass():
    import concourse.bacc as bacc
    import concourse.mybir as mybir
    from concourse.tile import TileContext

    f32 = mybir.dt.float32
    f32r = mybir.dt.float32r
    AX = mybir.AxisListType
    ALU = mybir.AluOpType
    ACTF = mybir.ActivationFunctionType

    nc = bacc.Bacc("TRN2", target_bir_lowering=False, debug=False)
    xt_d = nc.declare_dram_parameter("xt", [64, NMETA * SGC], f32r,
                                     isOutput=False)
    xa_d = nc.declare_dram_parameter("xa", [128, NG * TPG * DI], f32,
                                     isOutput=False)
    wl_d = nc.declare_dram_parameter("wl", [128, 128], f32r, isOutput=False)
    negk4_d = nc.declare_dram_parameter("negk4", [128, 1], f32,
                                        isOutput=False)
    c2d_d = nc.declare_dram_parameter("c2d", [128, TPG * FPAD], f32r,
                                      isOutput=False)
    yt_d = nc.declare_dram_parameter("yt", [128, NG * TPG * O], f32,
                                     isOutput=True)

    with TileContext(nc) as tc:
        with (
            tc.tile_pool(name="const", bufs=1) as cpool,
            tc.tile_pool(name="xin", bufs=3) as xpool,
            tc.tile_pool(name="work", bufs=3) as wpool,
            tc.tile_pool(name="sm", bufs=2) as spool,
            tc.tile_pool(name="ps_l", bufs=2, space="PSUM") as ps_l,
            tc.tile_pool(name="ps_g", bufs=3, space="PSUM") as ps_g,
        ):
            wl = cpool.tile([128, 128], f32r)
            nc.sync.dma_start(out=wl[:], in_=wl_d[:, :])
            negk4 = cpool.tile([128, 1], f32)
            nc.sync.dma_start(out=negk4[:], in_=negk4_d[:, :])
            c2d = cpool.tile([128, TPG * FPAD], f32r)
            nc.sync.dma_start(out=c2d[:], in_=c2d_d[:, :])

            for m in range(NMETA):
                # ---- load transposed x, square into lower half ----------
                xs = xpool.tile([128, SGC], f32r, tag="xs")
                nc.sync.dma_start(out=xs[0:64, :],
                                  in_=xt_d[:, m * SGC:(m + 1) * SGC])
                xa = xpool.tile([128, META * TPG * DI], f32, tag="xa")
                nc.sync.dma_start(
                    out=xa[:],
                    in_=xa_d[:, m * META * TPG * DI:(m + 1) * META * TPG * DI])
                nc.scalar.activation(xs[64:128, :], xs[0:64, :], ACTF.Square)
                # ---- M1: logits for the whole supergroup ----------------
                l = ps_l.tile([128, SGC], f32, tag="l")
                nc.tensor.matmul(l[:], lhsT=wl[:], rhs=xs[:],
                                 start=True, stop=True)
                # ---- strengths = exp(l - k) -----------------------------
                sst = wpool.tile([128, SGC], f32r, tag="sst")
                nc.scalar.activation(sst[:], l[:], ACTF.Exp,
                                     bias=negk4[:, 0:1], scale=1.0)

                u16 = spool.tile([128, META * TPG * O], f32, tag="u16")
                s16 = spool.tile([128, META * TPG], f32, tag="s16")
                e16 = spool.tile([128, META * TPG * O], f32, tag="e16")
                se16 = spool.tile([128, META * TPG], f32, tag="se16")
                o16 = spool.tile([128, META * TPG * O], f32, tag="o16")

                for q in range(META):
                    # -- M2: G = strengths-weighted consequent sums -------
                    # j-blocks at stride FPAD=256; cols [0:427) and
                    # [512:939) each sit inside one PSUM bank.
                    g4 = ps_g.tile([128, TPG * FPAD], f32, tag="g4")
                    lq = sst[:, q * 128:(q + 1) * 128]
                    w2 = FPAD + FS + 1      # f32r needs even free-dim
                    nc.tensor.matmul(g4[:, 0:w2], lhsT=lq,
                                     rhs=c2d[:, 0:w2],
                                     start=True, stop=True)
                    nc.tensor.matmul(g4[:, 512:512 + w2], lhsT=lq,
                                     rhs=c2d[:, 512:512 + w2],
                                     start=True, stop=True)
                    # -- P = G * xhat (bcast over o, unit-stride i) -------
                    g4v = g4[:].rearrange("p (j f) -> p j f",
                                          j=TPG)[:, :, 0:F].rearrange(
                        "p j (o i) -> p j o i", i=DI)
                    p4 = wpool.tile([128, TPG * F], f32, tag="p4")
                    p4v = p4[:].rearrange("p (j o i) -> p j o i",
                                          j=TPG, o=O)
                    xhv = xa[:, q * TPG * DI:(q + 1) * TPG * DI].rearrange(
                        "p (j i) -> p j i", i=DI).unsqueeze(2).broadcast_to(
                        [128, TPG, O, DI])
                    nc.vector.tensor_tensor(p4v, g4v, xhv, ALU.mult)
                    # -- U = sum_i P ; S from ones feature ----------------
                    nc.vector.tensor_reduce(
                        u16[:, q * TPG * O:(q + 1) * TPG * O].rearrange(
                            "p (j o) -> p j o", j=TPG),
                        p4v, axis=AX.X, op=ALU.add)
                    nc.vector.tensor_copy(
                        s16[:, q * TPG:(q + 1) * TPG],
                        g4[:].rearrange("p (j f) -> p j f",
                                        j=TPG)[:, :, F:F + 1].squeeze(2))

                # -- batched normalize + softmax over 16 tiles ------------
                nc.vector.tensor_scalar_add(s16[:], s16[:], EPS)
                nc.vector.reciprocal(s16[:], s16[:])
                u16v = u16[:].rearrange("p (g o) -> p g o", o=O)
                s16b = s16[:].unsqueeze(2).broadcast_to(
                    [128, META * TPG, O])
                nc.vector.tensor_tensor(u16v, u16v, s16b, ALU.mult)
                nc.scalar.activation(e16[:], u16[:], ACTF.Exp)
                nc.vector.tensor_reduce(
                    se16[:], e16[:].rearrange("p (g o) -> p g o", o=O),
                    axis=AX.X, op=ALU.add)
                nc.vector.reciprocal(se16[:], se16[:])
                se16b = se16[:].unsqueeze(2).broadcast_to(
                    [128, META * TPG, O])
                nc.vector.tensor_tensor(
                    o16[:].rearrange("p (g o) -> p g o", o=O),
                    e16[:].rearrange("p (g o) -> p g o", o=O),
                    se16b, ALU.mult)
                # -- store supergroup (contiguous) ------------------------
                nc.sync.dma_start(
                    out=yt_d[:, m * META * TPG * O:(m + 1) * META * TPG * O],
                    in_=o16[:])
    nc.compile()
    return nc


_NC_CACHE = None


def _prepare_in_maps(X, centers, sigmas, coeffs):
    wl, negk4, c2d = _build_constants(
        np.asarray(centers, np.float32),
        np.asarray(sigmas, np.float32),
        np.asarray(coeffs, np.float32))
    xt, xa = _prepare_x(X)
    in_maps = []
    for c in range(NCORES):
        in_maps.append({
            "xt": xt[c], "xa": xa[c],
            "wl": wl, "negk4": negk4, "c2d": c2d,
        })
    return in_maps


def kernel(X, centers, sigmas, coeffs):
    global _NC_CACHE
    from concourse import bass_utils

    if _NC_CACHE is None:
        _NC_CACHE = _build_bass()
    nc = _NC_CACHE

    in_maps = _prepare_in_maps(X, centers, sigmas, coeffs)
    res = bass_utils.run_bass_kernel_spmd(nc, in_maps, list(range(NCORES)))
    return _unpermute_out([r["yt"] for r in res.results])


# revision 19
# speedup vs baseline: 1.6700x; 1.3932x over previous
"""ANFIS forward kernel for Trainium2, 8-core data-parallel.

Model (reference):
    mf        = exp(-(x-c)^2 / (2 s^2))            # (N,R,D)
    strengths = prod_d mf                          # (N,R)
    norm      = strengths / (sum_r strengths + eps)
    wi        = X @ coeffs[r,:-1] + coeffs[r,-1]   # (N,R,O)
    out       = softmax_o( sum_r norm * wi )       # (N,O)

Kernel algebra (per row n):
    l_r   = sum_d [ 2*c*a*x - a*x^2 ] - k_r,   a = 1/(2 s^2), k_r = sum_d c^2 a
    s_r   = exp(l_r)                            (strengths)
    G_io  = sum_r s_r * Chat[r,i,o]             (Chat = [coeffs[:, :16]; bias row; ones])
    U_o   = sum_i xhat_i * G_io                 (xhat = [x, 1])
    S     = sum_r s_r  (via ones feature)
    out   = softmax_o( U / (S + eps) )

Layout: supergroups of 2048 rows = 16 tiles of 128 rows; j in 0..3 indexes
row-tiles within a 512-row group, q in 0..3 indexes groups in a supergroup.
  - Host supplies X pre-transposed (partition=(j,d)) so no PE transpose;
    ACT squares it into partitions 64..127.
  - M1 (f32r): one (128,512) matmul per supergroup -> logits l[(j,r),(q,p)].
  - exp on ACT with per-partition bias -k_r -> sst (f32r, SBUF).
  - M2 (f32r): per group, lhsT=sst column block, rhs=block-diag Chat with
    feature order f = o*17 + i (i innermost), padded to 256 per j-block.
  - DVE: P = G * xhat (broadcast over o, unit-stride i), reduce over i -> U.
  - Softmax batched over 16 tiles per supergroup.
All DRAM I/O is contiguous per partition; host does the (cheap) unpermutes.
"""

import numpy as np

N, D, R, O = 131072, 16, 32, 10
EPS = 1e-8
NCORES = 8
MC = N // NCORES          # rows per core = 16384
TPG = 4                   # tiles (of 128 rows) per group
GROUP = 128 * TPG         # 512 rows per group
NG = MC // GROUP          # 32 groups per core
META = 4                  # groups per supergroup
NMETA = NG // META        # 8 supergroups
SGC = META * 128          # matmul columns per supergroup (512)

DI = D + 1                # 17: x dims + ones
DIP = DI + 1              # 18: padded i-stride (keeps fp16 runs 4B-aligned)
F = DI * O                # 170 product features
OS = O + 1                # 11 o-blocks: 10 outputs + 1 strength-sum block
FS = OS * DIP             # 198 feature cols; col 196 = (o=10,i=16) = S ones
FPAD = 256                # per-j feature stride in G (bank-friendly)
SCALE = 16384.0           # strength rescale to dodge fp16 subnormals


def _build_constants(centers, sigmas, coeffs):
    a = 1.0 / (2.0 * sigmas.astype(np.float64) ** 2)          # (R,D)
    c = centers.astype(np.float64)

    # WL: lhsT for M1. out partition (j,r) = j*32+r ; rhs partition (s,j,d).
    wl = np.zeros((128, 128), np.float64)
    for j in range(TPG):
        for r in range(R):
            pi = j * R + r
            for d in range(D):
                wl[0 * 64 + j * 16 + d, pi] = 2.0 * c[r, d] * a[r, d]   # x
                wl[1 * 64 + j * 16 + d, pi] = -a[r, d]                  # x^2
    # +ln(SCALE) rescales strengths away from the fp16 subnormal zone;
    # the factor cancels in U/(S + SCALE*eps).
    negk = -(c * c * a).sum(axis=1) + np.log(SCALE)            # (R,)
    negk4 = np.tile(negk, TPG).reshape(128, 1)

    # Chat (R, FS) with feature order f = o*18 + i (i innermost, 1 pad col).
    # The S-sum feature sits at (o=10, i=16): the xhat broadcast there is
    # the ones column, so P[.,10,16] = S survives the mult+reduce unchanged.
    chat = np.zeros((R, FS), np.float64)
    chat.reshape(R, OS, DIP)[:, :O, :DI] = (
        coeffs.astype(np.float64).transpose(0, 2, 1).reshape(R, O, DI))
    chat[:, O * DIP + D] = 1.0
    # c2d (128, 4*FPAD): [(j,r), j*FPAD + f] = chat[r,f]
    c2d = np.zeros((128, TPG * FPAD), np.float64)
    for j in range(TPG):
        c2d[j * R:(j + 1) * R, j * FPAD:j * FPAD + FS] = chat
    return (wl.astype(np.float32), negk4.astype(np.float32),
            c2d.astype(np.float32))


def _prepare_x(X):
    """Per-core host-side layouts (pure permutation / padding)."""
    X = np.asarray(X, np.float32)
    # row = ((m*META + q)*TPG + j)*128 + p
    xv = X.reshape(NCORES, NMETA, META, TPG, 128, D)
    # XT[(j,d), m*512 + q*128 + p]
    xt = np.ascontiguousarray(
        xv.transpose(0, 3, 5, 1, 2, 4).reshape(NCORES, 64, NMETA * SGC))
    # xa[p, (m, q, j, i)] fp16, ones at i=16, zero pad at i=17
    xa = np.zeros((NCORES, 128, NMETA, META, TPG, DIP), np.float16)
    xa[..., :D] = xv.transpose(0, 4, 1, 2, 3, 5)
    xa[..., D] = 1.0
    xa = np.ascontiguousarray(xa.reshape(NCORES, 128, NG * TPG * DIP))
    return xt, xa


def _unpermute_out(yts):
    """yts: list of (128, NG*TPG*O) per core -> (N, O)."""
    y = np.stack(yts, axis=0).reshape(NCORES, 128, NMETA, META, TPG, O)
    y = y.transpose(0, 2, 3, 4, 1, 5).reshape(N, O)
    return np.ascontiguousarray(y)


def _build_bass():
    import concourse.bacc as bacc
    import concourse.mybir as mybir
    from concourse.tile import TileContext

    f32 = mybir.dt.float32
    f32r = mybir.dt.float32r
    f16 = mybir.dt.float16
    AX = mybir.AxisListType
    ALU = mybir.AluOpType
    ACTF = mybir.ActivationFunctionType

    nc = bacc.Bacc("TRN2", target_bir_lowering=False, debug=False)
    xt_d = nc.declare_dram_parameter("xt", [64, NMETA * SGC], f32r,
                                     isOutput=False)
    xa_d = nc.declare_dram_parameter("xa", [128, NG * TPG * DIP], f16,
                                     isOutput=False)
    wl_d = nc.declare_dram_parameter("wl", [128, 128], f32r, isOutput=False)
    negk4_d = nc.declare_dram_parameter("negk4", [128, 1], f32,
                                        isOutput=False)
    c2d_d = nc.declare_dram_parameter("c2d", [128, TPG * FPAD], f16,
                                      isOutput=False)
    yt_d = nc.declare_dram_parameter("yt", [128, NG * TPG * O], f32,
                                     isOutput=True)

    with TileContext(nc) as tc:
        with (
            tc.tile_pool(name="const", bufs=1) as cpool,
            tc.tile_pool(name="xin", bufs=3) as xpool,
            tc.tile_pool(name="work", bufs=3) as wpool,
            tc.tile_pool(name="sm", bufs=2) as spool,
            tc.tile_pool(name="ps_l", bufs=2, space="PSUM") as ps_l,
            tc.tile_pool(name="ps_g", bufs=3, space="PSUM") as ps_g,
        ):
            wl = cpool.tile([128, 128], f32r)
            nc.sync.dma_start(out=wl[:], in_=wl_d[:, :])
            negk4 = cpool.tile([128, 1], f32)
            nc.sync.dma_start(out=negk4[:], in_=negk4_d[:, :])
            c2d = cpool.tile([128, TPG * FPAD], f16)
            nc.sync.dma_start(out=c2d[:], in_=c2d_d[:, :])

            for m in range(NMETA):
                # ---- load transposed x, square into lower half ----------
                xs = xpool.tile([128, SGC], f32r, tag="xs")
                nc.sync.dma_start(out=xs[0:64, :],
                                  in_=xt_d[:, m * SGC:(m + 1) * SGC])
                xa = xpool.tile([128, META * TPG * DIP], f16, tag="xa")
                nc.sync.dma_start(
                    out=xa[:],
                    in_=xa_d[:,
                             m * META * TPG * DIP:(m + 1) * META * TPG * DIP])
                nc.scalar.activation(xs[64:128, :], xs[0:64, :], ACTF.Square)
                # ---- M1: logits for the whole supergroup ----------------
                l = ps_l.tile([128, SGC], f32, tag="l")
                nc.tensor.matmul(l[:], lhsT=wl[:], rhs=xs[:],
                                 start=True, stop=True)
                # ---- strengths = exp(l - k) -----------------------------
                sst = wpool.tile([128, SGC], f16, tag="sst")
                nc.scalar.activation(sst[:], l[:], ACTF.Exp,
                                     bias=negk4[:, 0:1], scale=1.0)

                u16 = spool.tile([128, META * TPG * OS], f32, tag="u16")
                e16 = spool.tile([128, META * TPG * OS], f32, tag="e16")
                se16 = spool.tile([128, META * TPG], f32, tag="se16")
                o16 = spool.tile([128, META * TPG * O], f32, tag="o16")

                for q in range(META):
                    # -- M2: G = strengths-weighted consequent sums -------
                    # j-blocks at stride FPAD=256; cols [0:454) and
                    # [512:966) each sit inside one PSUM bank.
                    g4 = ps_g.tile([128, TPG * FPAD], f32, tag="g4")
                    lq = sst[:, q * 128:(q + 1) * 128]
                    w2 = FPAD + FS
                    nc.tensor.matmul(g4[:, 0:w2], lhsT=lq,
                                     rhs=c2d[:, 0:w2],
                                     start=True, stop=True)
                    nc.tensor.matmul(g4[:, 512:512 + w2], lhsT=lq,
                                     rhs=c2d[:, 512:512 + w2],
                                     start=True, stop=True)
                    # -- evict G to fp16 SBUF (enables 2x DVE mult) -------
                    gh = wpool.tile([128, TPG * FS], f16, tag="gh")
                    nc.scalar.activation(
                        gh[:].rearrange("p (j f) -> p j f", j=TPG),
                        g4[:].rearrange("p (j f) -> p j f",
                                        j=TPG)[:, :, 0:FS],
                        ACTF.Copy)
                    # -- P = G * xhat (bcast over o, unit-stride i) -------
                    ghv = gh[:].rearrange("p (j o i) -> p j o i",
                                          j=TPG, o=OS)
                    p4 = wpool.tile([128, TPG * FS], f16, tag="p4")
                    p4v = p4[:].rearrange("p (j o i) -> p j o i",
                                          j=TPG, o=OS)
                    xhv = xa[:, q * TPG * DIP:(q + 1) * TPG * DIP].rearrange(
                        "p (j i) -> p j i", i=DIP).unsqueeze(2).broadcast_to(
                        [128, TPG, OS, DIP])
                    nc.vector.tensor_tensor(p4v, ghv, xhv, ALU.mult)
                    # -- U = sum_i P (S rides along in o-block 10) --------
                    nc.vector.tensor_reduce(
                        u16[:, q * TPG * OS:(q + 1) * TPG * OS].rearrange(
                            "p (j o) -> p j o", j=TPG),
                        p4v[:, :, :, 0:DI], axis=AX.X, op=ALU.add)

                # -- batched normalize + softmax over 16 tiles ------------
                u16v = u16[:].rearrange("p (g o) -> p g o", o=OS)
                sv = u16v[:, :, O:OS]
                nc.vector.tensor_scalar_add(sv, sv, EPS * SCALE)
                nc.vector.reciprocal(sv, sv)
                s16b = sv.broadcast_to([128, META * TPG, O])
                nc.vector.tensor_tensor(u16v[:, :, 0:O], u16v[:, :, 0:O],
                                        s16b, ALU.mult)
                nc.scalar.activation(e16[:], u16[:], ACTF.Exp)
                e16v = e16[:].rearrange("p (g o) -> p g o", o=OS)
                nc.vector.tensor_reduce(
                    se16[:], e16v[:, :, 0:O], axis=AX.X, op=ALU.add)
                nc.vector.reciprocal(se16[:], se16[:])
                se16b = se16[:].unsqueeze(2).broadcast_to(
                    [128, META * TPG, O])
                nc.vector.tensor_tensor(
                    o16[:].rearrange("p (g o) -> p g o", o=O),
                    e16v[:, :, 0:O],
                    se16b, ALU.mult)
                # -- store supergroup (contiguous) ------------------------
                nc.sync.dma_start(
                    out=yt_d[:, m * META * TPG * O:(m + 1) * META * TPG * O],
                    in_=o16[:])
    nc.compile()
    return nc


_NC_CACHE = None


def _prepare_in_maps(X, centers, sigmas, coeffs):
    wl, negk4, c2d = _build_constants(
        np.asarray(centers, np.float32),
        np.asarray(sigmas, np.float32),
        np.asarray(coeffs, np.float32))
    xt, xa = _prepare_x(X)
    in_maps = []
    for c in range(NCORES):
        in_maps.append({
            "xt": xt[c], "xa": xa[c],
            "wl": wl, "negk4": negk4, "c2d": c2d.astype(np.float16),
        })
    return in_maps


def kernel(X, centers, sigmas, coeffs):
    global _NC_CACHE
    from concourse import bass_utils

    if _NC_CACHE is None:
        _NC_CACHE = _build_bass()
    nc = _NC_CACHE

    in_maps = _prepare_in_maps(X, centers, sigmas, coeffs)
    res = bass_utils.run_bass_kernel_spmd(nc, in_maps, list(range(NCORES)))
    return _unpermute_out([r["yt"] for r in res.results])


# revision 21
# speedup vs baseline: 1.6766x; 1.0039x over previous
"""ANFIS forward kernel for Trainium2, 8-core data-parallel.

Model (reference):
    mf        = exp(-(x-c)^2 / (2 s^2))            # (N,R,D)
    strengths = prod_d mf                          # (N,R)
    norm      = strengths / (sum_r strengths + eps)
    wi        = X @ coeffs[r,:-1] + coeffs[r,-1]   # (N,R,O)
    out       = softmax_o( sum_r norm * wi )       # (N,O)

Kernel algebra (per row n):
    l_r   = sum_d [ 2*c*a*x - a*x^2 ] - k_r,   a = 1/(2 s^2), k_r = sum_d c^2 a
    s_r   = exp(l_r)                            (strengths)
    G_io  = sum_r s_r * Chat[r,i,o]             (Chat = [coeffs[:, :16]; bias row; ones])
    U_o   = sum_i xhat_i * G_io                 (xhat = [x, 1])
    S     = sum_r s_r  (via ones feature)
    out   = softmax_o( U / (S + eps) )

Layout: supergroups of 2048 rows = 16 tiles of 128 rows; j in 0..3 indexes
row-tiles within a 512-row group, q in 0..3 indexes groups in a supergroup.
  - Host supplies X pre-transposed (partition=(j,d)) so no PE transpose;
    ACT squares it into partitions 64..127.
  - M1 (f32r): one (128,512) matmul per supergroup -> logits l[(j,r),(q,p)].
  - exp on ACT with per-partition bias -k_r -> sst (f32r, SBUF).
  - M2 (f32r): per group, lhsT=sst column block, rhs=block-diag Chat with
    feature order f = o*17 + i (i innermost), padded to 256 per j-block.
  - DVE: P = G * xhat (broadcast over o, unit-stride i), reduce over i -> U.
  - Softmax batched over 16 tiles per supergroup.
All DRAM I/O is contiguous per partition; host does the (cheap) unpermutes.
"""

import numpy as np

N, D, R, O = 131072, 16, 32, 10
EPS = 1e-8
NCORES = 8
MC = N // NCORES          # rows per core = 16384
TPG = 4                   # tiles (of 128 rows) per group
GROUP = 128 * TPG         # 512 rows per group
NG = MC // GROUP          # 32 groups per core
META = 4                  # groups per supergroup
NMETA = NG // META        # 8 supergroups
SGC = META * 128          # matmul columns per supergroup (512)

DI = D + 1                # 17: x dims + ones
DIP = DI + 1              # 18: padded i-stride (keeps fp16 runs 4B-aligned)
F = DI * O                # 170 product features
OS = O + 1                # 11 o-blocks: 10 outputs + 1 strength-sum block
FS = OS * DIP             # 198 feature cols; col 196 = (o=10,i=16) = S ones
FPAD = 256                # per-j feature stride in G (bank-friendly)
SCALE = 16384.0           # strength rescale to dodge fp16 subnormals


def _build_constants(centers, sigmas, coeffs):
    a = 1.0 / (2.0 * sigmas.astype(np.float64) ** 2)          # (R,D)
    c = centers.astype(np.float64)

    # WL: lhsT for M1. out partition (j,r) = j*32+r ; rhs partition (s,j,d).
    wl = np.zeros((128, 128), np.float64)
    for j in range(TPG):
        for r in range(R):
            pi = j * R + r
            for d in range(D):
                wl[0 * 64 + j * 16 + d, pi] = 2.0 * c[r, d] * a[r, d]   # x
                wl[1 * 64 + j * 16 + d, pi] = -a[r, d]                  # x^2
    # +ln(SCALE) rescales strengths away from the fp16 subnormal zone;
    # the factor cancels in U/(S + SCALE*eps).
    negk = -(c * c * a).sum(axis=1) + np.log(SCALE)            # (R,)
    negk4 = np.tile(negk, TPG).reshape(128, 1)

    # Chat (R, FS) with feature order f = o*18 + i (i innermost, 1 pad col).
    # The S-sum feature sits at (o=10, i=16): the xhat broadcast there is
    # the ones column, so P[.,10,16] = S survives the mult+reduce unchanged.
    chat = np.zeros((R, FS), np.float64)
    chat.reshape(R, OS, DIP)[:, :O, :DI] = (
        coeffs.astype(np.float64).transpose(0, 2, 1).reshape(R, O, DI))
    chat[:, O * DIP + D] = 1.0
    # c2d (128, 4*FPAD): [(j,r), j*FPAD + f] = chat[r,f]
    c2d = np.zeros((128, TPG * FPAD), np.float64)
    for j in range(TPG):
        c2d[j * R:(j + 1) * R, j * FPAD:j * FPAD + FS] = chat
    return (wl.astype(np.float32), negk4.astype(np.float32),
            c2d.astype(np.float32))


def _prepare_x(X):
    """Per-core host-side layouts (pure permutation / padding)."""
    X = np.asarray(X, np.float32)
    # row = ((m*META + q)*TPG + j)*128 + p
    xv = X.reshape(NCORES, NMETA, META, TPG, 128, D)
    # XT[(j,d), m*512 + q*128 + p]
    xt = np.ascontiguousarray(
        xv.transpose(0, 3, 5, 1, 2, 4).reshape(NCORES, 64, NMETA * SGC))
    # xa[p, (m, q, j, i)] fp16, ones at i=16, zero pad at i=17
    xa = np.zeros((NCORES, 128, NMETA, META, TPG, DIP), np.float16)
    xa[..., :D] = xv.transpose(0, 4, 1, 2, 3, 5)
    xa[..., D] = 1.0
    xa = np.ascontiguousarray(xa.reshape(NCORES, 128, NG * TPG * DIP))
    return xt, xa


def _unpermute_out(yts):
    """yts: list of (128, NG*TPG*O) per core -> (N, O)."""
    y = np.stack(yts, axis=0).reshape(NCORES, 128, NMETA, META, TPG, O)
    y = y.transpose(0, 2, 3, 4, 1, 5).reshape(N, O)
    return np.ascontiguousarray(y)


def _build_bass():
    import concourse.bacc as bacc
    import concourse.mybir as mybir
    from concourse.tile import TileContext

    f32 = mybir.dt.float32
    f32r = mybir.dt.float32r
    f16 = mybir.dt.float16
    AX = mybir.AxisListType
    ALU = mybir.AluOpType
    ACTF = mybir.ActivationFunctionType

    nc = bacc.Bacc("TRN2", target_bir_lowering=False, debug=False)
    xt_d = nc.declare_dram_parameter("xt", [64, NMETA * SGC], f32r,
                                     isOutput=False)
    xa_d = nc.declare_dram_parameter("xa", [128, NG * TPG * DIP], f16,
                                     isOutput=False)
    wl_d = nc.declare_dram_parameter("wl", [128, 128], f32r, isOutput=False)
    negk4_d = nc.declare_dram_parameter("negk4", [128, 1], f32,
                                        isOutput=False)
    c2d_d = nc.declare_dram_parameter("c2d", [128, TPG * FPAD], f16,
                                      isOutput=False)
    yt_d = nc.declare_dram_parameter("yt", [128, NG * TPG * O], f32,
                                     isOutput=True)

    with TileContext(nc) as tc:
        with (
            tc.tile_pool(name="const", bufs=1) as cpool,
            tc.tile_pool(name="xin", bufs=3) as xpool,
            tc.tile_pool(name="work", bufs=3) as wpool,
            tc.tile_pool(name="sm", bufs=2) as spool,
            tc.tile_pool(name="ps_l", bufs=2, space="PSUM") as ps_l,
            tc.tile_pool(name="ps_g", bufs=3, space="PSUM") as ps_g,
        ):
            wl = cpool.tile([128, 128], f32r)
            nc.sync.dma_start(out=wl[:], in_=wl_d[:, :])
            negk4 = cpool.tile([128, 1], f32)
            nc.sync.dma_start(out=negk4[:], in_=negk4_d[:, :])
            c2d = cpool.tile([128, TPG * FPAD], f16)
            nc.sync.dma_start(out=c2d[:], in_=c2d_d[:, :])

            for m in range(NMETA):
                # ---- load transposed x, square into lower half ----------
                xs = xpool.tile([128, SGC], f32r, tag="xs")
                nc.sync.dma_start(out=xs[0:64, :],
                                  in_=xt_d[:, m * SGC:(m + 1) * SGC])
                xa = xpool.tile([128, META * TPG * DIP], f16, tag="xa")
                nc.sync.dma_start(
                    out=xa[:],
                    in_=xa_d[:,
                             m * META * TPG * DIP:(m + 1) * META * TPG * DIP])
                nc.scalar.activation(xs[64:128, :], xs[0:64, :], ACTF.Square)
                # ---- M1: logits for the whole supergroup ----------------
                l = ps_l.tile([128, SGC], f32, tag="l")
                nc.tensor.matmul(l[:], lhsT=wl[:], rhs=xs[:],
                                 start=True, stop=True)
                # ---- strengths = exp(l - k) -----------------------------
                sst = wpool.tile([128, SGC], f16, tag="sst")
                nc.scalar.activation(sst[:], l[:], ACTF.Exp,
                                     bias=negk4[:, 0:1], scale=1.0)

                u16 = spool.tile([128, META * TPG * OS], f32, tag="u16")
                e16 = spool.tile([128, META * TPG * OS], f32, tag="e16")
                se16 = spool.tile([128, META * TPG], f32, tag="se16")
                o16 = spool.tile([128, META * TPG * O], f32, tag="o16")

                gh = wpool.tile([128, META * TPG * FS], f16, tag="gh")
                for q in range(META):
                    # -- M2: G = strengths-weighted consequent sums -------
                    # j-blocks at stride FPAD=256; cols [0:454) and
                    # [512:966) each sit inside one PSUM bank.
                    g4 = ps_g.tile([128, TPG * FPAD], f32, tag="g4")
                    lq = sst[:, q * 128:(q + 1) * 128]
                    w2 = FPAD + FS
                    nc.tensor.matmul(g4[:, 0:w2], lhsT=lq,
                                     rhs=c2d[:, 0:w2],
                                     start=True, stop=True)
                    nc.tensor.matmul(g4[:, 512:512 + w2], lhsT=lq,
                                     rhs=c2d[:, 512:512 + w2],
                                     start=True, stop=True)
                    # -- evict G to fp16 SBUF (enables 2x DVE mult) -------
                    nc.scalar.activation(
                        gh[:, q * TPG * FS:(q + 1) * TPG * FS].rearrange(
                            "p (j f) -> p j f", j=TPG),
                        g4[:].rearrange("p (j f) -> p j f",
                                        j=TPG)[:, :, 0:FS],
                        ACTF.Copy)
                # -- P = G * xhat (bcast over o, unit-stride i), batched --
                NB = META * TPG
                ghv = gh[:].rearrange("p (g o i) -> p g o i", g=NB, o=OS)
                p4 = wpool.tile([128, NB * FS], f16, tag="p4")
                p4v = p4[:].rearrange("p (g o i) -> p g o i", g=NB, o=OS)
                xhv = xa[:].rearrange(
                    "p (g i) -> p g i", i=DIP).unsqueeze(2).broadcast_to(
                    [128, NB, OS, DIP])
                nc.vector.tensor_tensor(p4v, ghv, xhv, ALU.mult)
                # -- U = sum_i P (S rides along in o-block 10) ------------
                nc.vector.tensor_reduce(
                    u16[:].rearrange("p (g o) -> p g o", g=NB),
                    p4v[:, :, :, 0:DI], axis=AX.X, op=ALU.add)

                # -- batched normalize + softmax over 16 tiles ------------
                u16v = u16[:].rearrange("p (g o) -> p g o", o=OS)
                sv = u16v[:, :, O:OS]
                nc.vector.tensor_scalar_add(sv, sv, EPS * SCALE)
                nc.vector.reciprocal_approx_fast(sv, sv)
                s16b = sv.broadcast_to([128, META * TPG, O])
                nc.vector.tensor_tensor(u16v[:, :, 0:O], u16v[:, :, 0:O],
                                        s16b, ALU.mult)
                nc.scalar.activation(e16[:], u16[:], ACTF.Exp)
                e16v = e16[:].rearrange("p (g o) -> p g o", o=OS)
                nc.vector.tensor_reduce(
                    se16[:], e16v[:, :, 0:O], axis=AX.X, op=ALU.add)
                nc.vector.reciprocal_approx_fast(se16[:], se16[:])
                se16b = se16[:].unsqueeze(2).broadcast_to(
                    [128, META * TPG, O])
                nc.vector.tensor_tensor(
                    o16[:].rearrange("p (g o) -> p g o", o=O),
                    e16v[:, :, 0:O],
                    se16b, ALU.mult)
                # -- store supergroup (contiguous) ------------------------
                nc.sync.dma_start(
                    out=yt_d[:, m * META * TPG * O:(m + 1) * META * TPG * O],
                    in_=o16[:])
    nc.compile()
    return nc


_NC_CACHE = None


def _prepare_in_maps(X, centers, sigmas, coeffs):
    wl, negk4, c2d = _build_constants(
        np.asarray(centers, np.float32),
        np.asarray(sigmas, np.float32),
        np.asarray(coeffs, np.float32))
    xt, xa = _prepare_x(X)
    in_maps = []
    for c in range(NCORES):
        in_maps.append({
            "xt": xt[c], "xa": xa[c],
            "wl": wl, "negk4": negk4, "c2d": c2d.astype(np.float16),
        })
    return in_maps


def kernel(X, centers, sigmas, coeffs):
    global _NC_CACHE
    from concourse import bass_utils

    if _NC_CACHE is None:
        _NC_CACHE = _build_bass()
    nc = _NC_CACHE

    in_maps = _prepare_in_maps(X, centers, sigmas, coeffs)
    res = bass_utils.run_bass_kernel_spmd(nc, in_maps, list(range(NCORES)))
    return _unpermute_out([r["yt"] for r in res.results])
